# revision 1
# baseline (speedup 1.0000x reference)
"""Bass/Trainium2 SPMD kernel for the BipartiteGNN problem.

Architecture (8 NeuronCores, SPMD — one program, per-core data):
  - Nodes are sharded contiguously: core c owns rows [c*V, (c+1)*V), V=2500.
    Cores 0-3 own plant nodes, 4-7 pollinator nodes, so each core's encoder
    weights are just that side's MLP weights (passed as per-core inputs).
  - Within each core the owned nodes are sorted by in-degree (host-side
    permutation) so that groups of 128 destinations have near-uniform degree;
    each group's incoming messages are gathered with a single SWDGE
    dma_gather into an SBUF tile [128 dst, cap, 128 feat] and segment-summed
    with one strided vector reduce; padding slots point at a zeros row.
    Self-loop contributions are added from SBUF-resident epilogue tiles
    (never round-tripped through DRAM or gathered).
  - GCN normalization is factorized: each core computes only its own rows of
    hws = dinv * (h @ W) (node-major [V, 128]) and an AllGather concatenates
    them into the full gather table [N+1, 128] — so the per-layer matmul is
    never replicated and no separate h exchange is needed; after the segment
    sum the per-dst dinv scale and bias (+ReLU) are applied.

Everything graph-dependent is carried in int16/float32 input tensors; the
compiled program only depends on the per-group capacity schedule (cached).
"""

import os

import numpy as np

# Full-problem constants (hardcoded per harness contract).
N_NODES = 20000
N_EDGES = 640000
N_PLANTS = 10000
H = 128
N_CORES = 8
V_PER_CORE = N_NODES // N_CORES

KCH = int(os.environ.get("GNN_KCH", "64"))  # max gather blocks / instruction
REDUCE_MODE = os.environ.get("GNN_REDUCE_MODE", "stride")  # "stride" | "fold"


# ---------------------------------------------------------------------------
# Host-side planning: permutation, group capacities, gather index arrays.
# ---------------------------------------------------------------------------

class Plan:
    __slots__ = (
        "n", "v", "ncores", "ngroups", "caps", "base", "tk",
        "pi", "dinv", "idxs", "dinv_a", "dinv_b",
    )


def build_plan(src, dst, n, v, ncores):
    p = Plan()
    p.n, p.v, p.ncores = n, v, ncores
    G = (v + 127) // 128
    p.ngroups = G

    indeg = np.bincount(dst, minlength=n)
    deg = (indeg + 1).astype(np.float32)
    dinv = (1.0 / np.sqrt(deg)).astype(np.float32)
    p.dinv = dinv

    # Per-core in-degree sort (descending) of the owned rows.
    pi = np.empty(n, np.int64)
    for c in range(ncores):
        lo = c * v
        order = np.argsort(-indeg[lo:lo + v], kind="stable") + lo
        pi[lo:lo + v] = order
    inv_pi = np.empty(n, np.int64)
    inv_pi[pi] = np.arange(n)
    p.pi = pi

    indeg_perm = indeg[pi]

    # Group capacities: max member degree over all cores.
    caps = np.zeros(G, np.int64)
    for g in range(G):
        mx = 0
        for c in range(ncores):
            j0 = g * 128
            j1 = min(j0 + 128, v)
            blk = indeg_perm[c * v + j0: c * v + j1]
            if blk.size:
                mx = max(mx, int(blk.max()))
        caps[g] = max(mx, 1)
    p.caps = tuple(int(x) for x in caps)
    base = np.concatenate([[0], np.cumsum(caps)])
    p.base = base
    tk = int(base[-1]) * 128
    # token count must be a multiple of 16 (it is: *128)
    p.tk = tk

    # Table-row mapping for the split AllGather: the gather table is built by
    # two AGs (local rows [0:vh) and [vh:v)), each concatenating rank slices,
    # so permuted row r = c*v + j lands at table row:
    #   j < vh:  c*vh + j
    #   else:    ncores*vh + c*(v-vh) + (j-vh)
    gh = G // 2
    vh = 128 * gh

    def table_pos(r):
        c = r // v
        j = r % v
        return np.where(j < vh, c * vh + j,
                        ncores * vh + c * (v - vh) + (j - vh))

    # Token array per core: value = table row of source (or ZROW=n for pad).
    r_dst = inv_pi[dst]
    r_src = inv_pi[src]
    order_e = np.argsort(r_dst, kind="stable")
    rs = r_dst[order_e]
    ss = r_src[order_e]
    run_start = np.searchsorted(rs, rs, side="left")
    k_e = np.arange(len(rs)) - run_start
    c_e = rs // v
    j_e = rs % v
    g_e = j_e // 128
    p_e = j_e % 128
    pos_e = (base[g_e] + k_e) * 128 + p_e

    tokens = np.full((ncores, tk), n, np.int64)
    tokens[c_e, pos_e] = table_pos(ss)
    # (self contributions are added from SBUF-resident epilogue tiles,
    # not gathered)

    assert tokens.max() <= n and tokens.min() >= 0 and n < 32768
    t16 = tokens.astype(np.int16).reshape(ncores, tk // 16, 16).transpose(0, 2, 1)
    p.idxs = np.ascontiguousarray(np.tile(t16, (1, 8, 1)))  # [ncores,128,tk/16]

    # dinv in permuted order, laid out [128, blocks].
    pad = np.zeros((ncores, G * 128), np.float32)
    dv = dinv[pi]
    for c in range(ncores):
        pad[c, :v] = dv[c * v:(c + 1) * v]
    p.dinv_a = np.ascontiguousarray(
        pad.reshape(ncores * G, 128).T)          # [128, ncores*G] (same all cores)
    p.dinv_b = np.ascontiguousarray(
        pad.reshape(ncores, G, 128).transpose(0, 2, 1))  # [ncores, 128, G]
    return p


# ---------------------------------------------------------------------------
# Device program.
# ---------------------------------------------------------------------------

def build_program(n, v, ncores, caps, fake_ag=False):
    import concourse.bass as bass
    import concourse.bacc as bacc
    import concourse.mybir as mybir
    import concourse.tile as tile

    F32 = mybir.dt.float32
    I16 = mybir.dt.int16
    AF = mybir.ActivationFunctionType

    G = (v + 127) // 128
    base = [0]
    for cp in caps:
        base.append(base[-1] + cp)
    tk16 = base[-1] * 128 // 16

    nc = bacc.Bacc("TRN2", target_bir_lowering=False, debug=False,
                   num_devices=ncores)

    # --- I/O ---
    xT = nc.dram_tensor("xT", [H, v], F32, kind="ExternalInput")
    eW1 = nc.dram_tensor("eW1", [H, H], F32, kind="ExternalInput")
    eb1 = nc.dram_tensor("eb1", [H, 1], F32, kind="ExternalInput")
    eW2 = nc.dram_tensor("eW2", [H, H], F32, kind="ExternalInput")
    eb2 = nc.dram_tensor("eb2", [H, 1], F32, kind="ExternalInput")
    gW0 = nc.dram_tensor("gW0", [H, H], F32, kind="ExternalInput")
    gb0 = nc.dram_tensor("gb0", [H, 1], F32, kind="ExternalInput")
    gW1 = nc.dram_tensor("gW1", [H, H], F32, kind="ExternalInput")
    gb1 = nc.dram_tensor("gb1", [H, 1], F32, kind="ExternalInput")
    dinvB = nc.dram_tensor("dinvB", [128, G], F32, kind="ExternalInput")
    idxs = nc.dram_tensor("idxs", [128, tk16], I16, kind="ExternalInput")
    ident = nc.dram_tensor("ident", [128, 128], F32, kind="ExternalInput")
    houtT = nc.dram_tensor("houtT", [H, v], F32, kind="ExternalOutput")

    # --- internal DRAM ---
    # hws{L}_loc: this core's dinv-scaled (h @ W) rows; AllGathered into
    # hws{L}_full (node-major, +1 zeros row for gather padding). Split into
    # two tensors (rows [0:vh) / [vh:v)) so the first AllGather's dependency
    # covers only the first half of the producer's stores.
    ghalf = G // 2
    vh = 128 * ghalf
    if not (0 < vh < v):
        vh = v
    hws0_locA = nc.dram_tensor("hws0_locA", [vh, H], F32)
    hws1_locA = nc.dram_tensor("hws1_locA", [vh, H], F32)
    hws0_locB = (nc.dram_tensor("hws0_locB", [v - vh, H], F32)
                 if vh < v else None)
    hws1_locB = (nc.dram_tensor("hws1_locB", [v - vh, H], F32)
                 if vh < v else None)
    hws0_full = nc.dram_tensor("hws0_full", [n + 1, H], F32,
                               addr_space="Shared")
    hws1_full = nc.dram_tensor("hws1_full", [n + 1, H], F32,
                               addr_space="Shared")

    rg = [list(range(ncores))]

    with tile.TileContext(nc) as tc:
        with (
            tc.tile_pool(name="const", bufs=1) as cpool,
            tc.tile_pool(name="enc", bufs=3) as epool,
            tc.tile_pool(name="gth", bufs=4) as gpool,
            tc.tile_pool(name="stgb", bufs=4) as bpool,
            tc.tile_pool(name="selfp", bufs=1) as spool,
            tc.tile_pool(name="penc", bufs=2, space="PSUM") as penc,
            tc.tile_pool(name="pa", bufs=2, space="PSUM") as pa,
            tc.tile_pool(name="pt", bufs=2, space="PSUM") as pt,
        ):
            # ---- constants ----
            def cload(ap, shape, dt=F32, tag=None):
                t = cpool.tile(shape, dt, tag=tag)
                nc.sync.dma_start(t[:], ap)
                return t

            w1s = cload(eW1[:, :], [H, H], tag="w1")
            b1s = cload(eb1[:, :], [H, 1], tag="b1")
            w2s = cload(eW2[:, :], [H, H], tag="w2")
            b2s = cload(eb2[:, :], [H, 1], tag="b2")
            g0s = cload(gW0[:, :], [H, H], tag="g0")
            gb0s = cload(gb0[:, :], [H, 1], tag="gb0")
            g1s = cload(gW1[:, :], [H, H], tag="g1")
            gb1s = cload(gb1[:, :], [H, 1], tag="gb1")
            dBs = cload(dinvB[:, :], [128, G], tag="dB")
            idxs_sb = cload(idxs[:, :], [128, tk16], I16, tag="idx")
            ids = cload(ident[:, :], [128, 128], tag="id")
            zs = cpool.tile([1, H], F32, tag="zs")
            nc.vector.memset(zs[:], 0.0)
            nc.sync.dma_start(hws0_full[n:n + 1, :], zs[:])
            nc.sync.dma_start(hws1_full[n:n + 1, :], zs[:])

            self_tiles = {}

            def hw_scale_store(src_fm, j0, wb, W, hws_ab, lid, on_dve=False):
                """hws_loc[j0:j0+wb] = dinv * (h @ W) from feature-major h.
                The tile persists in SBUF as the group's self contribution."""
                g = j0 // 128
                pA = pa.tile([128, 128], F32, tag="pA")
                nc.tensor.matmul(pA[:wb, :], src_fm, W[:, :],
                                 start=True, stop=True)
                hb = spool.tile([128, 128], F32, tag=f"s{lid}g{g}")
                self_tiles[(lid, g)] = hb
                if wb < 128:
                    nc.vector.memset(hb[:, :], 0.0)
                if on_dve:
                    # head phase: ACT is the serial chain, DVE is idle
                    nc.vector.tensor_scalar_mul(hb[:wb, :], pA[:wb, :],
                                                dBs[:wb, g:g + 1])
                else:
                    nc.scalar.activation(hb[:wb, :], pA[:wb, :], AF.Copy,
                                         scale=dBs[:wb, g:g + 1])
                loc_a, loc_b = hws_ab
                if j0 < vh:
                    nc.sync.dma_start(loc_a[j0:j0 + wb, :], hb[:wb, :])
                else:
                    nc.sync.dma_start(loc_b[j0 - vh:j0 - vh + wb, :],
                                      hb[:wb, :])

            # ---- encoder (local nodes, feature-major) + hws0 ----
            for a0 in range(0, v, 256):
                w = min(256, v - a0)
                xt = epool.tile([128, 512], F32, tag="xt")
                nc.sync.dma_start(xt[:, :w], xT[:, a0:a0 + w])
                p1 = penc.tile([128, 512], F32, tag="p1")
                nc.tensor.matmul(p1[:, :w], w1s[:, :], xt[:, :w],
                                 start=True, stop=True)
                e1 = epool.tile([128, 512], F32, tag="e1")
                nc.scalar.activation(e1[:, :w], p1[:, :w], AF.Relu,
                                     bias=b1s[:, 0:1])
                p2 = penc.tile([128, 512], F32, tag="p2")
                nc.tensor.matmul(p2[:, :w], w2s[:, :], e1[:, :w],
                                 start=True, stop=True)
                e2 = epool.tile([128, 512], F32, tag="e2")
                nc.vector.tensor_scalar_add(e2[:, :w], p2[:, :w],
                                            b2s[:, 0:1])
                for j0 in range(0, w, 128):
                    wb = min(128, w - j0)
                    hw_scale_store(e2[:, j0:j0 + wb], a0 + j0, wb, g0s,
                                   (hws0_locA, hws0_locB), 0, on_dve=True)

            # Split AllGather: rows [0:vh) gathered separately from [vh:v) so
            # the first half can fire while the producer's second half still
            # computes. Table layout (see build_plan.table_pos) matches.
            def allgather(src_ab, dst):
                tb = 0
                for src in src_ab:
                    if src is None:
                        continue
                    w = src[:, :].shape[0]
                    if fake_ag:
                        # Dependency-correct stand-in: real AG runs on TOPSP
                        # SDMA (not these DMA engines), so model only the
                        # barrier, not the bytes.
                        nc.sync.dma_start(dst[tb:tb + 1, :], src[0:1, :])
                    else:
                        nc.gpsimd.collective_compute(
                            "AllGather", mybir.AluOpType.bypass,
                            replica_groups=rg,
                            ins=[src[:, :]],
                            outs=[dst[tb:tb + ncores * w, :]])
                    tb += ncores * w

            allgather((hws0_locA, hws0_locB), hws0_full)

            # ---- one GCN conv layer (gather + segment-sum + epilogue) ----
            def conv(hws, b_ap, relu, lid, outT=None, W_next=None,
                     hws_next=None):
                for g in range(G):
                    K = caps[g]
                    boff = base[g]
                    gt = gpool.tile([128, max(caps), 128], F32, tag="gt")
                    off = 0
                    while off < K:
                        kc = min(K - off, KCH)
                        nidx = 128 * kc
                        bo = boff + off
                        nc.gpsimd.dma_gather(
                            gt[:, off:off + kc, :], hws[:, :],
                            idxs_sb[:, 8 * bo:8 * (bo + kc)],
                            nidx, nidx, H, single_packet=False)
                        off += kc
                    ps = bpool.tile([128, 128], F32, tag="ps")
                    if K == 1:
                        nc.vector.tensor_copy(ps[:, :], gt[:, 0, :])
                    elif REDUCE_MODE == "stride":
                        nc.vector.tensor_reduce(
                            ps[:, :],
                            gt[:, :K, :].rearrange("p k f -> p f k"),
                            axis=mybir.AxisListType.X,
                            op=mybir.AluOpType.add)
                    else:
                        # contiguous fold-halving: result lands in block 0
                        kk = K
                        while kk > 1:
                            nk = (kk + 1) // 2
                            a = kk - nk
                            nc.vector.tensor_add(
                                gt[:, 0:a, :], gt[:, 0:a, :],
                                gt[:, nk:kk, :])
                            kk = nk
                        nc.vector.tensor_copy(ps[:, :], gt[:, 0, :])
                    nc.vector.tensor_add(ps[:, :], ps[:, :],
                                         self_tiles[(lid, g)][:, :])
                    sres = bpool.tile([128, 128], F32, tag="sres")
                    nc.vector.tensor_scalar_mul(sres[:, :], ps[:, :],
                                                dBs[:, g:g + 1])
                    pT = pt.tile([128, 128], F32, tag="pT")
                    nc.tensor.transpose(pT[:, :], sres[:, :], ids[:, :])
                    ob = bpool.tile([128, 128], F32, tag="ob")
                    nc.scalar.activation(ob[:, :], pT[:, :],
                                         AF.Relu if relu else AF.Identity,
                                         bias=b_ap[:, 0:1])
                    wg = min(128, v - g * 128)
                    if outT is not None:
                        nc.sync.dma_start(outT[:, g * 128:g * 128 + wg],
                                          ob[:, :wg])
                    else:
                        hw_scale_store(ob[:, :wg], g * 128, wg, W_next,
                                       hws_next, 1)

            conv(hws0_full, gb0s, relu=True, lid=0, W_next=g1s,
                 hws_next=(hws1_locA, hws1_locB))
            allgather((hws1_locA, hws1_locB), hws1_full)
            conv(hws1_full, gb1s, relu=False, lid=1, outT=houtT)

    nc.compile()
    return nc


# ---------------------------------------------------------------------------
# Host entry point.
# ---------------------------------------------------------------------------

_CACHE = {}


def _get_program(n, v, ncores, caps):
    key = (n, v, ncores, caps)
    prog = _CACHE.get(key)
    if prog is None:
        prog = build_program(n, v, ncores, caps)
        _CACHE[key] = prog
    return prog


def make_in_maps(x, plan, enc_w, gcn_w):
    """enc_w: (pW1,pb1,pW2,pb2,qW1,qb1,qW2,qb2); gcn_w: (gW0,gb0,gW1,gb1)."""
    n, v, ncores = plan.n, plan.v, plan.ncores
    pW1, pb1, pW2, pb2, qW1, qb1, qW2, qb2 = [
        np.ascontiguousarray(np.asarray(a, np.float32)) for a in enc_w]
    gW0, gb0, gW1, gb1 = [
        np.ascontiguousarray(np.asarray(a, np.float32)) for a in gcn_w]
    ident = np.eye(128, dtype=np.float32)
    plants_per_core = []
    in_maps = []
    nplant_cores = 0
    for c in range(ncores):
        rows = plan.pi[c * v:(c + 1) * v]
        is_plant = rows[0] < (n // 2)
        xTc = np.ascontiguousarray(x[rows].T.astype(np.float32))
        if is_plant:
            w1, b1, w2, b2 = pW1, pb1, pW2, pb2
        else:
            w1, b1, w2, b2 = qW1, qb1, qW2, qb2
        in_maps.append({
            "xT": xTc,
            "eW1": w1, "eb1": b1.reshape(H, 1),
            "eW2": w2, "eb2": b2.reshape(H, 1),
            "gW0": gW0, "gb0": gb0.reshape(H, 1),
            "gW1": gW1, "gb1": gb1.reshape(H, 1),
            "dinvB": plan.dinv_b[c],
            "idxs": plan.idxs[c],
            "ident": ident,
        })
        plants_per_core.append(is_plant)
    return in_maps


def assemble_output(results, plan):
    n, v = plan.n, plan.v
    out = np.empty((n, H), np.float32)
    for c in range(plan.ncores):
        out[plan.pi[c * v:(c + 1) * v]] = results[c]["houtT"].T
    return out


def kernel(**inputs):
    x = np.asarray(inputs["x"], np.float32)
    ei = np.asarray(inputs["edge_index"], np.int64)
    assert x.shape == (N_NODES, H) and ei.shape == (2, N_EDGES)
    assert int(inputs["num_plants"]) == N_PLANTS

    plan = build_plan(ei[0], ei[1], N_NODES, V_PER_CORE, N_CORES)
    nc = _get_program(N_NODES, V_PER_CORE, N_CORES, plan.caps)

    enc_w = (inputs["pW1"], inputs["pb1"], inputs["pW2"], inputs["pb2"],
             inputs["qW1"], inputs["qb1"], inputs["qW2"], inputs["qb2"])
    gcn_w = (inputs["gW0"], inputs["gb0"], inputs["gW1"], inputs["gb1"])
    in_maps = make_in_maps(x, plan, enc_w, gcn_w)

    from concourse.bass_utils import run_bass_kernel_spmd
    res = run_bass_kernel_spmd(nc, in_maps, list(range(N_CORES)))
    return assemble_output(res.results, plan)

